# revision 9
# baseline (speedup 1.0000x reference)
"""Trainium2 Bass kernel for a pre-LN transformer block (dense_transformer).

Shapes (hardcoded): B=16, N=1024, D=768, H=12, HD=64, HID=3072.
Sharding: data-parallel over batch across 8 NeuronCores (2 batch elements,
i.e. 2048 tokens, per core). Weights replicated; no collectives.

Per-core dataflow (activations kept on-chip, bf16 matmuls / fp32 stats):
  LN1 (token-major, fp32)   -> h1 bf16, PE-transposed to h1T [D, T]
  qkT [1536, T] = W_qk^T-stationary matmuls (LN gamma + 1/sqrt(hd) folded
                  into weights host-side)
  v_aug [T, 12*(64+1)]      = per-head V columns + a ones column (the ones
                  column makes the AV matmul also produce the softmax
                  denominator)
  per (batch, head):  scoresT[krow, qrow] = k^T.T @ q^T  (K=64 contraction)
                      attnT = Exp(scoresT)  (ScalarE, psum->sbuf bf16; exp is
                              max-free: |scores| is small by construction)
                      out^T[65, qrow] = [v | 1]^T-stationary @ attnT
                      row 64 = denominator -> reciprocal -> gpsimd partition
                      broadcast -> normalize rows 0..63 into attn_oT [D, T]
  proj (token-major out) + proj_b + residual x  -> y (fp32, SBUF)
  LN2 -> h2T;  fc1 (out-feature-major) + bias + exact Gelu -> gT
  fc2 (token-major out) + fc2_b + residual y -> output
"""

import numpy as np
import ml_dtypes

B, N, D = 16, 1024, 768
H = 12
HD = D // H
HID = 4 * D
EPS = 1e-6
NCORES = 8
BC = B // NCORES          # batch elements per core
T = BC * N                # tokens per core (2048)
NTT = T // 128            # token tiles (16)
KC = D // 128             # contraction chunks over D (6)
NTW = T // 512            # token windows of 512 (4)
NKT = N // 128            # key-row tiles per batch element (8)
NQW = N // 512            # query windows per batch element (2)
NOF1 = HID // 128         # fc1 output-feature tiles (24)

_PROG_CACHE = {}


def _build_program(has_qkv_bias):
    import concourse.bass as bass
    import concourse.mybir as mybir
    import concourse.tile as tile
    from concourse import bacc
    from concourse.masks import make_identity
    from contextlib import ExitStack

    F32 = mybir.dt.float32
    BF = mybir.dt.bfloat16
    AF = mybir.ActivationFunctionType
    ALU = mybir.AluOpType

    nc = bacc.Bacc("TRN2", target_bir_lowering=False, debug=False,
                   num_devices=NCORES)

    x_in = nc.dram_tensor("x", [T, D], F32, kind="ExternalInput").ap()
    qk_wT = nc.dram_tensor("qk_wT", [D, 2 * D], BF, kind="ExternalInput").ap()
    v_wT = nc.dram_tensor("v_wT", [D, D], BF, kind="ExternalInput").ap()
    proj_wT = nc.dram_tensor("proj_wT", [D, D], BF, kind="ExternalInput").ap()
    fc1_wT = nc.dram_tensor("fc1_wT", [D, HID], BF, kind="ExternalInput").ap()
    fc2_wT = nc.dram_tensor("fc2_wT", [HID, D], BF, kind="ExternalInput").ap()
    proj_b = nc.dram_tensor("proj_b", [D], F32, kind="ExternalInput").ap()
    fc1_b = nc.dram_tensor("fc1_b", [HID], F32, kind="ExternalInput").ap()
    fc2_b = nc.dram_tensor("fc2_b", [D], F32, kind="ExternalInput").ap()
    if has_qkv_bias:
        qk_bias = nc.dram_tensor("qk_b", [2 * D], F32, kind="ExternalInput").ap()
        v_bias = nc.dram_tensor("v_b", [D], F32, kind="ExternalInput").ap()
    y_out = nc.dram_tensor("y", [T, D], F32, kind="ExternalOutput").ap()

    with tile.TileContext(nc) as tc, ExitStack() as ctx:
        singles = ctx.enter_context(tc.tile_pool(name="singles", bufs=1))
        ident = singles.tile([128, 128], BF)
        make_identity(nc, ident)
        eps_t = singles.tile([128, 1], F32)
        nc.vector.memset(eps_t, EPS)
        fc1b_sb = singles.tile([128, NOF1], F32)
        nc.sync.dma_start(out=fc1b_sb, in_=fc1_b.rearrange("(a p) -> p a", p=128))
        def bcast128(ap1d):
            return bass.AP(tensor=ap1d.tensor, offset=ap1d.offset,
                           ap=[[0, 128]] + list(ap1d.ap))

        projb_sb = singles.tile([128, D], F32)
        nc.sync.dma_start(out=projb_sb, in_=bcast128(proj_b))
        fc2b_sb = singles.tile([128, D], F32)
        nc.sync.dma_start(out=fc2b_sb, in_=bcast128(fc2_b))
        if has_qkv_bias:
            qkb_sb = singles.tile([128, 2 * D // 128], F32)
            nc.sync.dma_start(out=qkb_sb,
                              in_=qk_bias.rearrange("(a p) -> p a", p=128))
            vb_sb = singles.tile([128, D], F32)
            nc.sync.dma_start(out=vb_sb, in_=bcast128(v_bias))

        # long-lived activation buffers (released manually, non-LIFO)
        h1T_pool = tc.alloc_tile_pool(name="h1T", bufs=KC, side="left")

        h1T = [h1T_pool.tile([128, T], BF, tag="h1T", name=f"h1T{i}") for i in range(KC)]

        # ---------- Phase A: LN1 + transpose ----------
        with tc.tile_pool(name="ln_in", bufs=3, side="right") as xin_pool, \
             tc.tile_pool(name="ln_tmp", bufs=8, side="right") as tmp_pool, \
             tc.tile_pool(name="ln_out", bufs=3, side="right") as hbf_pool, \
             tc.tile_pool(name="tr_psum", bufs=4, space="PSUM") as trp:
            for tt in range(NTT):
                xt = xin_pool.tile([128, D], F32, tag="xt")
                nc.sync.dma_start(out=xt, in_=x_in[tt * 128:(tt + 1) * 128, :])
                stats = tmp_pool.tile([128, 3, 6], F32, tag="stats")
                for sg in range(3):
                    nc.vector.bn_stats(stats[:, sg, :], xt[:, sg * 256:(sg + 1) * 256])
                mv = tmp_pool.tile([128, 2], F32, tag="mv")
                nc.vector.bn_aggr(mv, stats)
                rstd = tmp_pool.tile([128, 1], F32, tag="rstd")
                nc.scalar.activation(rstd, mv[:, 1:2], AF.Sqrt, bias=eps_t)
                nc.vector.reciprocal(rstd, rstd)
                negmr = tmp_pool.tile([128, 1], F32, tag="negmr")
                nc.vector.tensor_scalar(negmr, mv[:, 0:1], rstd, -1.0,
                                        ALU.mult, ALU.mult)
                h1 = hbf_pool.tile([128, D], BF, tag="h1")
                nc.vector.tensor_scalar(h1, xt, rstd, negmr, ALU.mult, ALU.add)
                for kc in range(KC):
                    pt = trp.tile([128, 128], BF, tag="pt")
                    nc.tensor.transpose(pt, h1[:, kc * 128:(kc + 1) * 128], ident)
                    nc.vector.tensor_copy(h1T[kc][:, tt * 128:(tt + 1) * 128], pt)

        qkT_pool = tc.alloc_tile_pool(name="qkT", bufs=12, side="right")
        vaug_pool = tc.alloc_tile_pool(name="vaug", bufs=NTT, side="right")
        qkT = [qkT_pool.tile([128, T], BF, tag="qkT", name=f"qkT{i}") for i in range(12)]
        v_aug = [vaug_pool.tile([128, H * (HD + 1)], BF, tag="vaug", name=f"vaug{i}")
                 for i in range(NTT)]

        # ---------- Phase B: QK^T and V ----------
        with tc.tile_pool(name="wqk", bufs=KC, side="left") as wqk_pool, \
             tc.tile_pool(name="wv", bufs=KC, side="left") as wv_pool, \
             tc.tile_pool(name="mm_psum", bufs=4, space="PSUM") as mmp:
            qkw_sb = [wqk_pool.tile([128, 2 * D], BF, tag="wqk", name=f"wqk{i}") for i in range(KC)]
            vw_sb = [wv_pool.tile([128, D], BF, tag="wv", name=f"wv{i}") for i in range(KC)]
            for kc in range(KC):
                nc.sync.dma_start(out=qkw_sb[kc],
                                  in_=qk_wT[kc * 128:(kc + 1) * 128, :])
                nc.sync.dma_start(out=vw_sb[kc],
                                  in_=v_wT[kc * 128:(kc + 1) * 128, :])
            # qkT[of][:, tw] = sum_kc qkw[kc][:, of-tile].T @ h1T[kc][:, tw]
            for of in range(12):
                for tw in range(NTW):
                    ps = mmp.tile([128, 512], F32, tag="mm")
                    for kc in range(KC):
                        nc.tensor.matmul(
                            ps,
                            lhsT=qkw_sb[kc][:, of * 128:(of + 1) * 128],
                            rhs=h1T[kc][:, tw * 512:(tw + 1) * 512],
                            start=(kc == 0), stop=(kc == KC - 1))
                    if has_qkv_bias:
                        nc.vector.tensor_scalar(
                            qkT[of][:, tw * 512:(tw + 1) * 512], ps,
                            qkb_sb[:, of:of + 1], None, ALU.add)
                    else:
                        nc.vector.tensor_copy(
                            qkT[of][:, tw * 512:(tw + 1) * 512], ps)
            # v (token-major) with per-head ones column appended
            for tt in range(NTT):
                for n0, nsz in ((0, 512), (512, 256)):
                    ps = mmp.tile([128, nsz], F32, tag="mm")
                    for kc in range(KC):
                        nc.tensor.matmul(
                            ps,
                            lhsT=h1T[kc][:, tt * 128:(tt + 1) * 128],
                            rhs=vw_sb[kc][:, n0:n0 + nsz],
                            start=(kc == 0), stop=(kc == KC - 1))
                    # scatter head-blocks of 64 into stride-65 layout
                    nh = nsz // HD
                    h0 = n0 // HD
                    dst = v_aug[tt][:, h0 * (HD + 1):(h0 + nh) * (HD + 1)] \
                        .rearrange("p (n c) -> p n c", c=HD + 1)[:, :, 0:HD]
                    psr = ps.rearrange("p (n c) -> p n c", c=HD)
                    if has_qkv_bias:
                        src_b = vb_sb[:, n0:n0 + nsz] \
                            .rearrange("p (n c) -> p n c", c=HD)
                        nc.vector.tensor_add(dst, psr, src_b)
                    else:
                        nc.vector.tensor_copy(dst, psr)
                ones_dst = v_aug[tt].rearrange(
                    "p (n c) -> p n c", c=HD + 1)[:, :, HD:HD + 1]
                nc.vector.memset(ones_dst, 1.0)

        h1T_pool.release()
        oT_pool = tc.alloc_tile_pool(name="oT", bufs=KC, side="left")
        attn_oT = [oT_pool.tile([128, T], BF, tag="oT", name=f"oT{i}") for i in range(KC)]

        # ---------- Phase C: attention ----------
        with tc.tile_pool(name="attnT", bufs=2 * NKT, side="right") as at_pool, \
             tc.tile_pool(name="sc_psum", bufs=4, space="PSUM") as scp, \
             tc.tile_pool(name="av_psum", bufs=2, space="PSUM") as avp, \
             tc.tile_pool(name="rec", bufs=4, side="right") as rec_pool, \
             tc.tile_pool(name="recb", bufs=4, side="right") as recb_pool:
            for b in range(BC):
                for h in range(H):
                    q_t = qkT[h // 2]
                    k_t = qkT[6 + h // 2]
                    r0 = (h % 2) * HD
                    ats = [at_pool.tile([128, N], BF, tag="at", name=f"at{i}") for i in range(NKT)]
                    for kt in range(NKT):
                        for qw in range(NQW):
                            ps = scp.tile([128, 512], F32, tag="sc")
                            nc.tensor.matmul(
                                ps,
                                lhsT=k_t[r0:r0 + HD,
                                         b * N + kt * 128:b * N + (kt + 1) * 128],
                                rhs=q_t[r0:r0 + HD,
                                        b * N + qw * 512:b * N + (qw + 1) * 512],
                                start=True, stop=True)
                            nc.scalar.activation(
                                ats[kt][:, qw * 512:(qw + 1) * 512], ps, AF.Exp)
                    o_t = attn_oT[h // 2]
                    for qw in range(NQW):
                        pav = avp.tile([HD + 1, 512], F32, tag="av")
                        for kt in range(NKT):
                            nc.tensor.matmul(
                                pav,
                                lhsT=v_aug[b * NKT + kt][:, h * (HD + 1):(h + 1) * (HD + 1)],
                                rhs=ats[kt][:, qw * 512:(qw + 1) * 512],
                                start=(kt == 0), stop=(kt == NKT - 1))
                        rec = rec_pool.tile([1, 512], F32, tag="rec")
                        nc.vector.reciprocal(rec, pav[HD:HD + 1, :])
                        recb = recb_pool.tile([HD, 512], F32, tag="recb")
                        nc.gpsimd.partition_broadcast(recb, rec)
                        nc.vector.tensor_mul(
                            o_t[r0:r0 + HD, b * N + qw * 512:b * N + (qw + 1) * 512],
                            pav[0:HD, :], recb)

        vaug_pool.release()
        qkT_pool.release()
        y_pool = tc.alloc_tile_pool(name="y", bufs=NTT, side="right")
        y_sb = [y_pool.tile([128, D], F32, tag="y", name=f"ysb{i}") for i in range(NTT)]

        # ---------- Phase D: proj + residual ----------
        with tc.tile_pool(name="wproj", bufs=KC, side="left") as wp_pool, \
             tc.tile_pool(name="x_res", bufs=3, side="left") as xr_pool, \
             tc.tile_pool(name="mm_psum2", bufs=4, space="PSUM") as mmp:
            pw_sb = [wp_pool.tile([128, D], BF, tag="wp", name=f"wp{i}") for i in range(KC)]
            for kc in range(KC):
                nc.sync.dma_start(out=pw_sb[kc],
                                  in_=proj_wT[kc * 128:(kc + 1) * 128, :])
            for tt in range(NTT):
                xt = xr_pool.tile([128, D], F32, tag="xres")
                nc.sync.dma_start(out=xt, in_=x_in[tt * 128:(tt + 1) * 128, :])
                for n0, nsz in ((0, 512), (512, 256)):
                    ps = mmp.tile([128, nsz], F32, tag="mm2")
                    for kc in range(KC):
                        nc.tensor.matmul(
                            ps,
                            lhsT=attn_oT[kc][:, tt * 128:(tt + 1) * 128],
                            rhs=pw_sb[kc][:, n0:n0 + nsz],
                            start=(kc == 0), stop=(kc == KC - 1))
                    # y = psum + x + proj_b
                    nc.vector.tensor_add(y_sb[tt][:, n0:n0 + nsz], ps,
                                         xt[:, n0:n0 + nsz])
                nc.vector.tensor_add(y_sb[tt], y_sb[tt], projb_sb)

        oT_pool.release()
        h2T_pool = tc.alloc_tile_pool(name="h2T", bufs=KC, side="left")
        h2T = [h2T_pool.tile([128, T], BF, tag="h2T", name=f"h2T{i}") for i in range(KC)]

        # ---------- Phase E: LN2 + transpose ----------
        with tc.tile_pool(name="ln2_tmp", bufs=8, side="right") as tmp_pool, \
             tc.tile_pool(name="ln2_out", bufs=3, side="right") as hbf_pool, \
             tc.tile_pool(name="tr_psum2", bufs=4, space="PSUM") as trp:
            for tt in range(NTT):
                yt = y_sb[tt]
                stats = tmp_pool.tile([128, 3, 6], F32, tag="stats2")
                for sg in range(3):
                    nc.vector.bn_stats(stats[:, sg, :], yt[:, sg * 256:(sg + 1) * 256])
                mv = tmp_pool.tile([128, 2], F32, tag="mv2")
                nc.vector.bn_aggr(mv, stats)
                rstd = tmp_pool.tile([128, 1], F32, tag="rstd2")
                nc.scalar.activation(rstd, mv[:, 1:2], AF.Sqrt, bias=eps_t)
                nc.vector.reciprocal(rstd, rstd)
                negmr = tmp_pool.tile([128, 1], F32, tag="negmr2")
                nc.vector.tensor_scalar(negmr, mv[:, 0:1], rstd, -1.0,
                                        ALU.mult, ALU.mult)
                h2 = hbf_pool.tile([128, D], BF, tag="h2")
                nc.vector.tensor_scalar(h2, yt, rstd, negmr, ALU.mult, ALU.add)
                for kc in range(KC):
                    pt = trp.tile([128, 128], BF, tag="pt2")
                    nc.tensor.transpose(pt, h2[:, kc * 128:(kc + 1) * 128], ident)
                    nc.vector.tensor_copy(h2T[kc][:, tt * 128:(tt + 1) * 128], pt)

        # ---------- Phase F: MLP ----------
        with tc.tile_pool(name="wfc1", bufs=KC, side="right") as w1_pool, \
             tc.tile_pool(name="wfc2", bufs=NOF1, side="right") as w2_pool, \
             tc.tile_pool(name="g", bufs=NOF1 + 2, side="right") as g_pool, \
             tc.tile_pool(name="out", bufs=3, side="right") as out_pool, \
             tc.tile_pool(name="fc1_psum", bufs=4, space="PSUM") as f1p, \
             tc.tile_pool(name="fc2_psum", bufs=3, space="PSUM") as f2p:
            w1_sb = [w1_pool.tile([128, HID], BF, tag="w1", name=f"w1_{i}") for i in range(KC)]
            for kc in range(KC):
                nc.sync.dma_start(out=w1_sb[kc],
                                  in_=fc1_wT[kc * 128:(kc + 1) * 128, :])
            w2_sb = [w2_pool.tile([128, D], BF, tag="w2", name=f"w2_{i}") for i in range(NOF1)]
            for c in range(NOF1):
                nc.sync.dma_start(out=w2_sb[c],
                                  in_=fc2_wT[c * 128:(c + 1) * 128, :])
            for tw in range(NTW):
                gts = [g_pool.tile([128, 512], BF, tag="g", name=f"g{i}") for i in range(NOF1)]
                for of in range(NOF1):
                    ps = f1p.tile([128, 512], F32, tag="f1")
                    for kc in range(KC):
                        nc.tensor.matmul(
                            ps,
                            lhsT=w1_sb[kc][:, of * 128:(of + 1) * 128],
                            rhs=h2T[kc][:, tw * 512:(tw + 1) * 512],
                            start=(kc == 0), stop=(kc == KC - 1))
                    nc.scalar.activation(gts[of], ps, AF.Gelu,
                                         bias=fc1b_sb[:, of:of + 1])
                for tl in range(4):
                    tt = tw * 4 + tl
                    o_sb = out_pool.tile([128, D], F32, tag="o")
                    for n0, nsz in ((0, 512), (512, 256)):
                        ps = f2p.tile([128, nsz], F32, tag="f2")
                        for c in range(NOF1):
                            nc.tensor.matmul(
                                ps,
                                lhsT=gts[c][:, tl * 128:(tl + 1) * 128],
                                rhs=w2_sb[c][:, n0:n0 + nsz],
                                start=(c == 0), stop=(c == NOF1 - 1))
                        nc.vector.tensor_add(o_sb[:, n0:n0 + nsz], ps,
                                             y_sb[tt][:, n0:n0 + nsz])
                    nc.vector.tensor_add(o_sb, o_sb, fc2b_sb)
                    nc.sync.dma_start(out=y_out[tt * 128:(tt + 1) * 128, :],
                                      in_=o_sb)
        y_pool.release()
        h2T_pool.release()

    nc.compile()
    return nc


def _get_program(has_qkv_bias):
    key = bool(has_qkv_bias)
    if key not in _PROG_CACHE:
        _PROG_CACHE[key] = _build_program(key)
    return _PROG_CACHE[key]


def kernel(x, qkv_w, proj_w, proj_b, fc1_w, fc1_b, fc2_w, fc2_b,
           norm1_g, norm1_b, norm2_g, norm2_b):
    from concourse.bass_utils import run_bass_kernel_spmd

    x = np.asarray(x, dtype=np.float32)
    qkv_w = np.asarray(qkv_w, dtype=np.float32)
    proj_w = np.asarray(proj_w, dtype=np.float32)
    fc1_w = np.asarray(fc1_w, dtype=np.float32)
    fc2_w = np.asarray(fc2_w, dtype=np.float32)

    bf = ml_dtypes.bfloat16
    scale = HD ** (-0.5)

    # fold LN1 gamma into qkv_w columns; LN1 beta becomes a qkv bias.
    w_eff = qkv_w * np.asarray(norm1_g, np.float32)[None, :]
    b_eff = qkv_w @ np.asarray(norm1_b, np.float32)
    # fold the attention scale into q
    w_eff = w_eff.copy()
    w_eff[:D] *= scale
    b_eff = b_eff.copy()
    b_eff[:D] *= scale
    has_qkv_bias = bool(np.any(b_eff != 0.0))

    qk_wT = np.ascontiguousarray(w_eff[:2 * D].T, dtype=bf)
    v_wT = np.ascontiguousarray(w_eff[2 * D:].T, dtype=bf)
    proj_wT = np.ascontiguousarray(proj_w.T, dtype=bf)
    # fold LN2 gamma into fc1_w columns; LN2 beta into fc1 bias.
    fc1_eff = fc1_w * np.asarray(norm2_g, np.float32)[None, :]
    fc1_b_eff = np.asarray(fc1_b, np.float32) + fc1_w @ np.asarray(norm2_b, np.float32)
    fc1_wT = np.ascontiguousarray(fc1_eff.T, dtype=bf)
    fc2_wT = np.ascontiguousarray(fc2_w.T, dtype=bf)

    shared = {
        "qk_wT": qk_wT, "v_wT": v_wT, "proj_wT": proj_wT,
        "fc1_wT": fc1_wT, "fc2_wT": fc2_wT,
        "proj_b": np.ascontiguousarray(proj_b, np.float32),
        "fc1_b": np.ascontiguousarray(fc1_b_eff, np.float32),
        "fc2_b": np.ascontiguousarray(fc2_b, np.float32),
    }
    if has_qkv_bias:
        shared["qk_b"] = np.ascontiguousarray(b_eff[:2 * D], np.float32)
        shared["v_b"] = np.ascontiguousarray(b_eff[2 * D:], np.float32)

    in_maps = []
    for c in range(NCORES):
        xc = np.ascontiguousarray(
            x[c * BC:(c + 1) * BC].reshape(T, D), dtype=np.float32)
        in_maps.append({"x": xc, **shared})

    nc = _get_program(has_qkv_bias)
    res = run_bass_kernel_spmd(nc, in_maps, core_ids=list(range(NCORES)))

    out = np.empty((B, N, D), dtype=np.float32)
    for c in range(NCORES):
        out[c * BC:(c + 1) * BC] = res.results[c]["y"].reshape(BC, N, D)
    return out
